# revision 7
# baseline (speedup 1.0000x reference)
"""BitLinear (ternary-weight / int8-activation quantized linear) on 8 trn2 NeuronCores.

Math (must match the jax reference bit-for-bit up to fp32 rounding):
    eta   = clip(max|x|  along k, 1e-5)            per row      [B*S, 1]
    x_q   = round(x * 127 / eta)   in [-127, 127]  (round-half-even)
    gamma = clip(mean|w|, 1e-5)                    scalar
    w_q   = round(clip(w / gamma, -1, 1))          in {-1,0,1}
    out   = (x_q @ w_q^T) * (eta/127 * gamma) + bias

Key numeric fact: x_q and w_q are small integers exactly representable in
bf16, and the PE accumulates in fp32, so the bf16 matmul is EXACT.
Rounding uses the fp32 magic-number trick: (t + 1.5*2^23) - 1.5*2^23 == rint(t)
(round-half-even, matching jnp.round).

Sharding: data-parallel over rows of x (16384 rows -> 2048 rows/core),
weight + bias replicated.  No collectives.
"""

import os

import numpy as np
import ml_dtypes

import concourse.bass as bass
import concourse.bacc as bacc
import concourse.mybir as mybir
import concourse.tile as tile
from concourse.bass_utils import run_bass_kernel_spmd

P = 128
K = 2048            # contraction dim
N = 2048            # out features
M_CORE = 2048       # rows per core
KT = K // P         # 16 k-tiles
NT = N // P         # 16 weight row-tiles
MT = M_CORE // P    # 16 x row-tiles per core
NBLK = N // 512     # 4 psum-width blocks
MBLK = MT // 4      # 4 m-blocks (512 rows) for the x transpose round-trip
N_CORES = 8
C_MAGIC = 12582912.0          # 1.5 * 2**23 : fp32 rint magic constant
INV_NK = 1.0 / (N * K)        # for mean|w|

F32 = mybir.dt.float32
BF16 = mybir.dt.bfloat16
ALU = mybir.AluOpType
AXIS = mybir.AxisListType
ACTF = mybir.ActivationFunctionType


def _build_program():
    nc = bacc.Bacc("TRN2", target_bir_lowering=False, debug=False)

    x_d = nc.dram_tensor("x", [M_CORE, K], F32, kind="ExternalInput").ap()
    w_d = nc.dram_tensor("weight", [N, K], F32, kind="ExternalInput").ap()
    b_d = nc.dram_tensor("bias", [P, N], F32, kind="ExternalInput").ap()
    out_d = nc.dram_tensor("out", [M_CORE, N], F32, kind="ExternalOutput").ap()
    xq_rt_d = nc.dram_tensor("xq_rt", [M_CORE, K], BF16).ap()  # internal round-trip
    ident_d = nc.inline_tensor(
        np.eye(P, dtype=ml_dtypes.bfloat16), name="ident128"
    ).ap()

    from contextlib import ExitStack

    with tile.TileContext(nc) as tc, ExitStack() as ctx:
        consts = ctx.enter_context(tc.tile_pool(name="consts", bufs=1))
        stats = ctx.enter_context(tc.tile_pool(name="stats", bufs=1))
        wstage = ctx.enter_context(tc.tile_pool(name="wstage", bufs=2))
        wqst = ctx.enter_context(tc.tile_pool(name="wqst", bufs=2))
        wqT_p = ctx.enter_context(tc.tile_pool(name="wqT", bufs=KT))
        xstage = ctx.enter_context(tc.tile_pool(name="xstage", bufs=2))
        xqst = ctx.enter_context(tc.tile_pool(name="xqst", bufs=2))
        xqT_p = ctx.enter_context(tc.tile_pool(name="xqT", bufs=KT))
        outst = ctx.enter_context(tc.tile_pool(name="outst", bufs=3))
        ps_tr = ctx.enter_context(
            tc.tile_pool(name="pstr", bufs=2, space=bass.MemorySpace.PSUM)
        )
        ps_mm = ctx.enter_context(
            tc.tile_pool(name="psmm", bufs=5, space=bass.MemorySpace.PSUM)
        )

        # ---- constants ----
        ident_sb = consts.tile([P, P], BF16)
        nc.sync.dma_start(ident_sb[:], ident_d[:, :])
        ones128 = consts.tile([P, P], F32)
        nc.vector.memset(ones128[:], 1.0)
        bias_bc = consts.tile([P, N], F32)
        nc.sync.dma_start(bias_bc[:], b_d[:, :])

        # ---- stats tiles ----
        eta_raw = stats.tile([P, MT], F32)   # per-row abs-max, unclipped
        eta_all = stats.tile([P, MT], F32)   # clipped
        inv_eta = stats.tile([P, MT], F32)
        qs_all = stats.tile([P, MT], F32)    # 127/eta
        osc_all = stats.tile([P, MT], F32)   # eta*gamma/127
        wparts = stats.tile([P, NT], F32)
        wsum = stats.tile([P, 1], F32)
        gamma = stats.tile([P, 1], F32)
        inv_g = stats.tile([P, 1], F32)

        # persistent operand tiles (k on partitions)
        wqT = [wqT_p.tile([P, N], BF16, tag="wqT", name=f"wqT{kt}") for kt in range(KT)]
        xqT = [xqT_p.tile([P, M_CORE], BF16, tag="xqT", name=f"xqT{kt}") for kt in range(KT)]

        # ---- phase W1: stream w, fused abs-add reduce (for gamma) ----
        def w1_iter(nt):
            t = wstage.tile([P, K], F32, tag="w")
            nc.sync.dma_start(t[:], w_d[nt * P:(nt + 1) * P, :])
            nc.vector.tensor_reduce(
                wparts[:, nt:nt + 1], t[:], axis=AXIS.X, op=ALU.add,
                apply_absolute_value=True,
            )

        # ---- phase X: stream x, eta, quantize, store bf16 for transpose RT ----
        def x_iter(mt):
            t = xstage.tile([P, K], F32, tag="x")
            nc.gpsimd.dma_start(t[:], x_d[mt * P:(mt + 1) * P, :])
            nc.vector.tensor_reduce(
                eta_raw[:, mt:mt + 1], t[:], axis=AXIS.X, op=ALU.max,
                apply_absolute_value=True,
            )
            nc.vector.tensor_scalar(
                eta_all[:, mt:mt + 1], eta_raw[:, mt:mt + 1],
                scalar1=1e-5, scalar2=None, op0=ALU.max,
            )
            nc.vector.reciprocal(inv_eta[:, mt:mt + 1], eta_all[:, mt:mt + 1])
            nc.vector.tensor_scalar(
                qs_all[:, mt:mt + 1], inv_eta[:, mt:mt + 1],
                scalar1=127.0, scalar2=None, op0=ALU.mult,
            )
            # t*qs + C : fp32 output rounds to the integer grid (RNE)
            nc.vector.tensor_scalar(
                t[:], t[:],
                scalar1=qs_all[:, mt:mt + 1], scalar2=C_MAGIC,
                op0=ALU.mult, op1=ALU.add,
            )
            q = xqst.tile([P, K], BF16, tag="xq")
            nc.vector.tensor_scalar(
                q[:], t[:], scalar1=C_MAGIC, scalar2=None, op0=ALU.subtract,
            )
            nc.gpsimd.dma_start(xq_rt_d[mt * P:(mt + 1) * P, :], q[:])

        def xqT_load(mb):
            # transposed read-back of the bf16 x_q m-block via the DMA xbar
            r0, r1 = mb * 512, (mb + 1) * 512
            for kt in range(KT):
                nc.scalar.dma_start_transpose(
                    xqT[kt][:, r0:r1],
                    xq_rt_d[r0:r1, kt * P:(kt + 1) * P],
                )

        # ---- phase W2: stream w again, quantize, PE-transpose into wqT ----
        def w2_iter(nt):
            t = wstage.tile([P, K], F32, tag="w")
            nc.sync.dma_start(t[:], w_d[nt * P:(nt + 1) * P, :])
            # (w * 1/gamma) clipped to [-1,1], then +C/-C rint
            nc.vector.tensor_scalar(
                t[:], t[:], scalar1=inv_g[:, :], scalar2=-1.0,
                op0=ALU.mult, op1=ALU.max,
            )
            nc.vector.tensor_scalar(
                t[:], t[:], scalar1=1.0, scalar2=C_MAGIC,
                op0=ALU.min, op1=ALU.add,
            )
            q = wqst.tile([P, K], BF16, tag="wq")
            nc.vector.tensor_scalar(
                q[:], t[:], scalar1=C_MAGIC, scalar2=None, op0=ALU.subtract,
            )
            for kt in range(KT):
                pt = ps_tr.tile([P, P], BF16, tag="ptr")
                nc.tensor.transpose(pt[:], q[:, kt * P:(kt + 1) * P], ident_sb[:])
                nc.scalar.copy(wqT[kt][:, nt * P:(nt + 1) * P], pt[:])

        # ================= emission order (schedule priority) =================
        for nt in range(NT):
            w1_iter(nt)
        for mt in range(4):
            x_iter(mt)

        # gamma chain: ones.T @ wsum puts the full |w| sum on all 128 partitions
        nc.vector.tensor_reduce(wsum[:], wparts[:], axis=AXIS.X, op=ALU.add)
        pg = ps_mm.tile([P, 1], F32, tag="psg", name="psg", bufs=1)
        nc.tensor.matmul(pg[:], ones128[:, :], wsum[:])
        nc.vector.tensor_scalar(
            gamma[:], pg[:], scalar1=INV_NK, scalar2=1e-5,
            op0=ALU.mult, op1=ALU.max,
        )
        nc.vector.reciprocal(inv_g[:], gamma[:])

        xqT_load(0)
        for nt in range(NT):
            w2_iter(nt)
        for mt in range(4, 8):
            x_iter(mt)
        xqT_load(1)
        for mt in range(8, 12):
            x_iter(mt)
        xqT_load(2)
        for mt in range(12, 16):
            x_iter(mt)
        xqT_load(3)

        # ---- matmul + dequant + store ----
        for mt in range(MT):
            # osc = eta * gamma / 127   (per-partition scalar for this m-tile)
            nc.vector.tensor_scalar(
                osc_all[:, mt:mt + 1], eta_all[:, mt:mt + 1],
                scalar1=gamma[:, :], scalar2=1.0 / 127.0,
                op0=ALU.mult, op1=ALU.mult,
            )
            pss = [ps_mm.tile([P, 512], F32, tag="psmm", name=f"ps{mt}_{nb}") for nb in range(NBLK)]
            ms = slice(mt * P, (mt + 1) * P)
            for kt in range(KT):
                for nb in range(NBLK):
                    nc.tensor.matmul(
                        pss[nb][:],
                        xqT[kt][:, ms],
                        wqT[kt][:, nb * 512:(nb + 1) * 512],
                        start=(kt == 0),
                        stop=(kt == KT - 1),
                    )
            for nb in range(NBLK):
                o = outst.tile([P, 512], F32, tag="o")
                nc.vector.scalar_tensor_tensor(
                    o[:], pss[nb][:], osc_all[:, mt:mt + 1],
                    bias_bc[:, nb * 512:(nb + 1) * 512],
                    op0=ALU.mult, op1=ALU.add,
                )
                nc.scalar.dma_start(
                    out_d[ms, nb * 512:(nb + 1) * 512], o[:]
                )
    nc.compile()
    return nc


_NC_CACHE = None
LAST_EXEC_NS = None


def _get_nc():
    global _NC_CACHE
    if _NC_CACHE is None:
        _NC_CACHE = _build_program()
    return _NC_CACHE


def _make_in_maps(x, weight, bias):
    xf = np.ascontiguousarray(np.asarray(x, dtype=np.float32).reshape(-1, K))
    w = np.ascontiguousarray(np.asarray(weight, dtype=np.float32))
    b = np.ascontiguousarray(
        np.broadcast_to(np.asarray(bias, dtype=np.float32).reshape(1, N), (P, N))
    )
    assert xf.shape[0] == N_CORES * M_CORE
    return [
        {
            "x": xf[c * M_CORE:(c + 1) * M_CORE],
            "weight": w,
            "bias": b,
        }
        for c in range(N_CORES)
    ]


def kernel(x, weight, bias):
    global LAST_EXEC_NS
    nc = _get_nc()
    in_maps = _make_in_maps(x, weight, bias)
    trace = bool(int(os.environ.get("BITLINEAR_TRACE", "0")))
    res = run_bass_kernel_spmd(nc, in_maps, list(range(N_CORES)), trace=trace)
    LAST_EXEC_NS = res.exec_time_ns
    out = np.concatenate([res.results[c]["out"] for c in range(N_CORES)], axis=0)
    return out.reshape(np.asarray(x).shape[:-1] + (N,)).astype(np.float32)


# revision 10
# speedup vs baseline: 1.1843x; 1.1843x over previous
"""BitLinear (ternary-weight / int8-activation quantized linear) on 8 trn2 NeuronCores.

Math (matches the jax reference up to fp32 rounding):
    eta   = clip(max|x| along k, 1e-5)             per row
    x_q   = round(x * 127 / eta)    in [-127,127]  (round-half-even)
    gamma = clip(mean|w|, 1e-5)                    scalar
    w_q   = round(clip(w / gamma, -1, 1))          in {-1,0,1}
    out   = (x_q @ w_q^T) * (eta/127 * gamma) + bias

x_q / w_q are small integers exactly representable in bf16 and the PE
accumulates in fp32, so the bf16 matmul is EXACT.  Rounding uses the fp32
magic-number trick  rint(t) = (t + 1.5*2^23) - 1.5*2^23  (round-half-even).

Sharding: data-parallel over rows of x (16384 -> 2048 rows/core), weight+bias
replicated.  Per-core schedule:
  phase W: stream w once (both HWDGE queues), fused |w| reduce -> gamma,
           quantize, PE-transpose into k-major wqT (SBUF resident, bf16)
  phase X: stream x, per-row eta, quantize, round-trip x_q through DRAM;
           m-block 0 transposed on PE, blocks 1-3 via DMA-xbar transposed
           loads that overlap the matmul phase
  phase MM: 1024 bf16 matmuls (k-contiguous per m-tile), ACT dequant-scale
           from PSUM, DVE bias add, stores on the sync queue
"""

import os
from contextlib import ExitStack

import numpy as np
import ml_dtypes

import concourse.bass as bass
import concourse.bacc as bacc
import concourse.mybir as mybir
import concourse.tile as tile
from concourse.bass_utils import run_bass_kernel_spmd

P = 128
K = 2048
N = 2048
M_CORE = 2048
KT = K // P          # 16
NT = N // P          # 16
MT = M_CORE // P     # 16
NBLK = N // 512      # 4
N_CORES = 8
C_MAGIC = 12582912.0     # 1.5 * 2**23
INV_NK = 1.0 / (N * K)

F32 = mybir.dt.float32
BF16 = mybir.dt.bfloat16
ALU = mybir.AluOpType
AXIS = mybir.AxisListType
ACTF = mybir.ActivationFunctionType


def _build_program():
    nc = bacc.Bacc("TRN2", target_bir_lowering=False, debug=False)

    x_d = nc.dram_tensor("x", [M_CORE, K], F32, kind="ExternalInput").ap()
    w_d = nc.dram_tensor("weight", [N, K], F32, kind="ExternalInput").ap()
    b_d = nc.dram_tensor("bias", [P, N], F32, kind="ExternalInput").ap()
    out_d = nc.dram_tensor("out", [M_CORE, N], F32, kind="ExternalOutput").ap()
    xq_rt_d = nc.dram_tensor("xq_rt", [M_CORE, K], BF16).ap()
    ident_d = nc.inline_tensor(
        np.eye(P, dtype=ml_dtypes.bfloat16), name="ident128"
    ).ap()

    with tile.TileContext(nc) as tc, ExitStack() as ctx:
        consts = ctx.enter_context(tc.tile_pool(name="consts", bufs=1))
        stats = ctx.enter_context(tc.tile_pool(name="stats", bufs=1))
        wqT_p = ctx.enter_context(tc.tile_pool(name="wqT", bufs=1))
        ps_tr = ctx.enter_context(
            tc.tile_pool(name="pstr", bufs=2, space=bass.MemorySpace.PSUM)
        )
        ps_mm = ctx.enter_context(
            tc.tile_pool(name="psmm", bufs=5, space=bass.MemorySpace.PSUM)
        )

        # ---- constants / stats ----
        ident_sb = consts.tile([P, P], BF16)
        nc.sync.dma_start(ident_sb[:], ident_d[:, :])
        ones128 = consts.tile([P, P], F32)
        nc.vector.memset(ones128[:], 1.0)

        eta_raw = stats.tile([P, MT], F32)
        eta_all = stats.tile([P, MT], F32)
        inv_eta = stats.tile([P, MT], F32)
        qs_all = stats.tile([P, MT], F32)
        osc_all = stats.tile([P, MT], F32)
        wparts = stats.tile([P, NT], F32)
        wsum = stats.tile([P, 1], F32)
        gamma = stats.tile([P, 1], F32)
        inv_g = stats.tile([P, 1], F32)

        # k-major quantized operands, single big tiles:
        # layout [128 k-part, kt*2048 + col]
        wqT_all = wqT_p.tile([P, KT * N], BF16)
        wqT_3d = wqT_all[:].rearrange("p (t n) -> p t n", t=KT)

        # =================== phase W ===================
        with tc.tile_pool(name="wstage", bufs=NT) as wstage, \
             tc.tile_pool(name="wqst", bufs=3) as wqst:
            wtiles = []
            for nt in range(NT):
                t = wstage.tile([P, K], F32, tag="w", name=f"w{nt}")
                eng = nc.sync if nt % 2 == 0 else nc.scalar
                eng.dma_start(t[:], w_d[nt * P:(nt + 1) * P, :])
                nc.vector.tensor_reduce(
                    wparts[:, nt:nt + 1], t[:], axis=AXIS.X, op=ALU.add,
                    apply_absolute_value=True,
                )
                wtiles.append(t)

            # gamma = clip(sum/NK, 1e-5); ones-matmul broadcasts the
            # partition sum to all 128 partitions in one shot
            nc.vector.tensor_reduce(wsum[:], wparts[:], axis=AXIS.X, op=ALU.add)
            pg = ps_mm.tile([P, 1], F32, tag="psg", name="psg", bufs=1)
            nc.tensor.matmul(pg[:], ones128[:, :], wsum[:])
            nc.vector.tensor_scalar(
                gamma[:], pg[:], scalar1=INV_NK, scalar2=1e-5,
                op0=ALU.mult, op1=ALU.max,
            )
            nc.vector.reciprocal(inv_g[:], gamma[:])

            for nt in range(NT):
                t = wtiles[nt]
                # t = w / gamma  (ACT), then clip (DVE), then rint -> bf16 (DVE)
                nc.scalar.activation(
                    t[:], t[:], ACTF.Copy, bias=0.0, scale=inv_g[:, :]
                )
                nc.vector.tensor_scalar(
                    t[:], t[:], scalar1=1.0, scalar2=-1.0,
                    op0=ALU.min, op1=ALU.max,
                )
                q = wqst.tile([P, K], BF16, tag="wq", name=f"wq{nt}")
                nc.vector.tensor_scalar(
                    q[:], t[:], scalar1=C_MAGIC, scalar2=C_MAGIC,
                    op0=ALU.add, op1=ALU.subtract,
                )
                # PE-transpose the 16 k-blocks, 4 per PSUM tile, one strided
                # ACT copy per group into wqT_all
                for g in range(4):
                    pt = ps_tr.tile([P, 512], BF16, tag="ptr", name=f"wt{nt}_{g}")
                    for j in range(4):
                        kt = g * 4 + j
                        nc.tensor.transpose(
                            pt[:, j * P:(j + 1) * P],
                            q[:, kt * P:(kt + 1) * P],
                            ident_sb[:],
                        )
                    nc.scalar.copy(
                        wqT_3d[:, g * 4:(g + 1) * 4, nt * P:(nt + 1) * P],
                        pt[:].rearrange("p (j n) -> p j n", j=4),
                    )

        # =================== phase X ===================
        with tc.tile_pool(name="xqT", bufs=1) as xqT_p, \
             tc.tile_pool(name="xstage", bufs=4) as xstage, \
             tc.tile_pool(name="xqst", bufs=4) as xqst, \
             tc.tile_pool(name="bias_p", bufs=1) as bias_p, \
             tc.tile_pool(name="outst", bufs=3) as outst:
            xqT_all = xqT_p.tile([P, KT * M_CORE], BF16)
            xqT_3d = xqT_all[:].rearrange("p (t m) -> p t m", t=KT)
            bias_bc = bias_p.tile([P, N], F32)
            nc.sync.dma_start(bias_bc[:], b_d[:, :])

            xq_tiles = {}

            def x_iter(mt, store_rt):
                t = xstage.tile([P, K], F32, tag="x", name=f"x{mt}")
                nc.gpsimd.dma_start(t[:], x_d[mt * P:(mt + 1) * P, :])
                nc.vector.tensor_reduce(
                    eta_raw[:, mt:mt + 1], t[:], axis=AXIS.X, op=ALU.max,
                    apply_absolute_value=True,
                )
                nc.vector.tensor_scalar(
                    eta_all[:, mt:mt + 1], eta_raw[:, mt:mt + 1],
                    scalar1=1e-5, scalar2=None, op0=ALU.max,
                )
                nc.vector.reciprocal(inv_eta[:, mt:mt + 1], eta_all[:, mt:mt + 1])
                nc.vector.tensor_scalar(
                    qs_all[:, mt:mt + 1], inv_eta[:, mt:mt + 1],
                    scalar1=127.0, scalar2=None, op0=ALU.mult,
                )
                # t = x*qs + C  (ACT; fp32 output rounds to the integer grid)
                nc.scalar.activation(
                    t[:], t[:], ACTF.Copy, bias=C_MAGIC,
                    scale=qs_all[:, mt:mt + 1],
                )
                q = xqst.tile([P, K], BF16, tag="xq", name=f"xq{mt}")
                nc.vector.tensor_scalar(
                    q[:], t[:], scalar1=C_MAGIC, scalar2=None, op0=ALU.subtract,
                )
                xq_tiles[mt] = q
                if store_rt:
                    nc.gpsimd.dma_start(xq_rt_d[mt * P:(mt + 1) * P, :], q[:])

            # m-block 0: quantize then PE-transpose directly (bridges the PE
            # gap between the w transposes and the first matmuls)
            for mt in range(4):
                x_iter(mt, store_rt=False)
                q = xq_tiles[mt]
                for g in range(4):
                    pt = ps_tr.tile([P, 512], BF16, tag="ptr", name=f"xt{mt}_{g}")
                    for j in range(4):
                        kt = g * 4 + j
                        nc.tensor.transpose(
                            pt[:, j * P:(j + 1) * P],
                            q[:, kt * P:(kt + 1) * P],
                            ident_sb[:],
                        )
                    nc.scalar.copy(
                        xqT_3d[:, g * 4:(g + 1) * 4, mt * P:(mt + 1) * P],
                        pt[:].rearrange("p (j m) -> p j m", j=4),
                    )

            def xbar_load(r0, r1):
                for kt in range(KT):
                    nc.scalar.dma_start_transpose(
                        xqT_3d[:, kt, r0:r1],
                        xq_rt_d[r0:r1, kt * P:(kt + 1) * P],
                    )

            for mt in range(4, 8):
                x_iter(mt, store_rt=True)
            xbar_load(512, 1024)
            for mt in range(8, 16):
                x_iter(mt, store_rt=True)
            xbar_load(1024, 2048)

            # =================== phase MM ===================
            for mt in range(MT):
                nc.vector.tensor_scalar(
                    osc_all[:, mt:mt + 1], eta_all[:, mt:mt + 1],
                    scalar1=gamma[:, :], scalar2=1.0 / 127.0,
                    op0=ALU.mult, op1=ALU.mult,
                )
                pss = [
                    ps_mm.tile([P, 512], F32, tag="psmm", name=f"ps{mt}_{nb}")
                    for nb in range(NBLK)
                ]
                for kt in range(KT):
                    lhsT = xqT_3d[:, kt, mt * P:(mt + 1) * P]
                    for nb in range(NBLK):
                        nc.tensor.matmul(
                            pss[nb][:],
                            lhsT,
                            wqT_3d[:, kt, nb * 512:(nb + 1) * 512],
                            start=(kt == 0),
                            stop=(kt == KT - 1),
                        )
                for nb in range(NBLK):
                    o = outst.tile([P, 512], F32, tag="o", name=f"o{mt}_{nb}")
                    # dequant scale on ACT (psum -> sbuf), bias add on DVE
                    nc.scalar.activation(
                        o[:], pss[nb][:], ACTF.Copy, bias=0.0,
                        scale=osc_all[:, mt:mt + 1],
                    )
                    nc.vector.tensor_tensor(
                        o[:], o[:], bias_bc[:, nb * 512:(nb + 1) * 512],
                        op=ALU.add,
                    )
                    nc.sync.dma_start(
                        out_d[mt * P:(mt + 1) * P, nb * 512:(nb + 1) * 512], o[:]
                    )
    nc.compile()
    return nc


_NC_CACHE = None
LAST_EXEC_NS = None


def _get_nc():
    global _NC_CACHE
    if _NC_CACHE is None:
        _NC_CACHE = _build_program()
    return _NC_CACHE


def _make_in_maps(x, weight, bias):
    xf = np.ascontiguousarray(np.asarray(x, dtype=np.float32).reshape(-1, K))
    w = np.ascontiguousarray(np.asarray(weight, dtype=np.float32))
    b = np.ascontiguousarray(
        np.broadcast_to(np.asarray(bias, dtype=np.float32).reshape(1, N), (P, N))
    )
    assert xf.shape[0] == N_CORES * M_CORE
    return [
        {
            "x": xf[c * M_CORE:(c + 1) * M_CORE],
            "weight": w,
            "bias": b,
        }
        for c in range(N_CORES)
    ]


def kernel(x, weight, bias):
    global LAST_EXEC_NS
    nc = _get_nc()
    in_maps = _make_in_maps(x, weight, bias)
    trace = bool(int(os.environ.get("BITLINEAR_TRACE", "0")))
    res = run_bass_kernel_spmd(nc, in_maps, list(range(N_CORES)), trace=trace)
    LAST_EXEC_NS = res.exec_time_ns
    out = np.concatenate([res.results[c]["out"] for c in range(N_CORES)], axis=0)
    return out.reshape(np.asarray(x).shape[:-1] + (N,)).astype(np.float32)
